# revision 80
# baseline (speedup 1.0000x reference)
"""Causal self-attention (GQA + RoPE + QK-norm) Trainium2 Bass kernel.

Sharding: 8 cores = 4 batches x 2 head-groups.  Core c -> batch c//2,
q heads (c%2)*8..+8, kv heads (c%2)*2..+2.  wproj is row-sharded, so each
core emits a partial (T, C) output; the host sums the two partials per batch.

Device-side layout (per core):
  - x fed pre-transposed (xT, [C, T]) bf16; cos|sin fed as one [T, 128] f32.
    The warmup DMA schedule is hand-ordered so the first matmuls start ~4us
    in, and chunks 0-1 run co-outer so PE keeps pace with weight arrival.
  - Phase A: QKV projection accumulates into one 3-bank PSUM tile per token
    chunk; RoPE + qk-norm run token-major on DVE (norm factors are exact:
    RoPE preserves per-head L2 norms, and 1/sqrt(D) is folded into the
    rsqrt via 1/sqrt(ss + D*eps)).  Each chunk's postproc is split: partA
    frees the PSUM bank (V evac + the four RoPE PSUM reads), partB (norm +
    transpose) is deferred one chunk so the PE never waits on the DVE norm
    chain.  qT/kT transposes run on the DMA engines (XBAR dma_transpose),
    entirely off the PE.
  - Phase B: scores computed transposed (scoresT[tk, tq]); the causal mask
    is a triangular-NEG matmul accumulated into the PSUM bank before the
    score matmul (has_written semantics).  exp runs with bias -4 so fp16
    accumulators cannot overflow, writing fp16 p tiles that feed the p@v
    matmul directly.  Softmax denominators accumulate on DVE (pacc, fp16)
    and are reduced+broadcast across partitions by one GpSimd
    partition_all_reduce per head; normalization is one DVE recip+mul.
    All PE work is emitted from a single globally software-pipelined
    stream (y-matmuls lag scores, per-head epilogues lag further) so the
    PE never drains at head/tile boundaries.
  - Phase C (output projection) is interleaved into phase B: tile t's
    projection matmuls are paced into tile t+1's attention stream, filling
    the PE gaps left by the exp critical path.  The final tile's projection
    is split into two 4-head passes (partials staged in SBUF) so half of it
    overlaps the last attention tile.
"""

import numpy as np
import ml_dtypes
from contextlib import ExitStack

import concourse.bass as bass
import concourse.bass_isa as bass_isa
import concourse.mybir as mybir
import concourse.tile as tile
from concourse import bacc
from concourse.bass_utils import run_bass_kernel_spmd
from concourse.masks import make_identity

BF16 = mybir.dt.bfloat16
F16 = mybir.dt.float16
F32 = mybir.dt.float32
AF = mybir.ActivationFunctionType
ALU = mybir.AluOpType

B, T, C = 4, 2048, 2048
H, KV, D = 16, 4, 128
HG, KVG = H // 2, KV // 2          # per-core q heads (8), kv heads (2)
QC, KC = HG * D, KVG * D           # 1024, 256
P = 128
TOKCH = T // P                     # 16 token chunks
NREP = H // KV                     # 4
EPS = 1e-5
NEG = -1.0e5                       # additive causal mask (exp -> 0)
EXPB = -4.0                        # exp bias: pt <= e^{11.32-4} ~ 1.5e3; a
                                   # 16-chunk fp16 pacc sum stays < 2.5e4

PHASES = ("A", "B")
PSC_BUFS, PSY_BUFS, PSO_BUFS = 3, 2, 3
CPACE_F = 1.3
LAG_Y_CFG, LAG_E_CFG = 4, 7


def _build():
    nc = bacc.Bacc("TRN2", target_bir_lowering=False, debug=False, num_devices=8)
    xt = nc.dram_tensor("xt", [C, T], BF16, kind="ExternalInput")
    wq = nc.dram_tensor("wq", [C, QC], BF16, kind="ExternalInput")
    wkv = nc.dram_tensor("wkv", [C, 2 * KC], BF16, kind="ExternalInput")
    wp = nc.dram_tensor("wp", [QC, C], BF16, kind="ExternalInput")
    cs = nc.dram_tensor("cs", [T, P], F32, kind="ExternalInput")
    out = nc.dram_tensor("out", [T, C], F32, kind="ExternalOutput")

    with tile.TileContext(nc) as tc, ExitStack() as ctx:
        singles = ctx.enter_context(tc.tile_pool(name="singles", bufs=1))
        spool = ctx.enter_context(tc.tile_pool(name="sa", bufs=2))
        qpool = ctx.enter_context(tc.tile_pool(name="sb2", bufs=2))
        qkpool = ctx.enter_context(tc.tile_pool(name="qkp", bufs=3))
        a_stack = ExitStack()   # pools freed after phase A (wp reuses space)
        xpool = a_stack.enter_context(tc.tile_pool(name="xp", bufs=5))

        xr = xt.rearrange("(co p) t -> p co t", p=P)
        xtiles = {}

        def load_x(t2):
            xtile = xpool.tile([P, C // P, 2 * P], BF16, tag="xt")
            nc.sync.dma_start(xtile, xr[:, :, t2 * 2 * P:(t2 + 1) * 2 * P])
            xtiles[t2] = xtile

        # ---- weights (A-scoped pool so phase C's wp reuses the space);
        # DMA issue order interleaves x prefetch with per-co weight blocks
        # so PE can start at ~3us and never starves during chunk 0-3 ----
        wqkvp = a_stack.enter_context(tc.tile_pool(name="wqkv", bufs=1))
        wq_sb = wqkvp.tile([P, C // P, QC], BF16)
        wkv_sb = wqkvp.tile([P, C // P, 2 * KC], BF16)
        cs_sb = singles.tile([P, TOKCH, P], F32)
        wqr = wq.rearrange("(co p) q -> p co q", p=P)
        wkvr = wkv.rearrange("(co p) q -> p co q", p=P)
        csr = cs.rearrange("(tc p) d -> p tc d", p=P)

        # warmup schedule: smallest pieces that unblock the first matmuls,
        # then weight blocks paced ahead of the prologue's co consumption
        x0 = xpool.tile([P, C // P, 2 * P], BF16, tag="xt")
        xtiles[0] = x0
        nc.sync.dma_start(x0[:, 0:8, :], xr[:, 0:8, 0:2 * P])
        nc.sync.dma_start(wq_sb[:, 0:1, :], wqr[:, 0:1, :])
        nc.sync.dma_start(wkv_sb[:, 0:1, :], wkvr[:, 0:1, :])
        nc.sync.dma_start(wq_sb[:, 1:2, :], wqr[:, 1:2, :])
        nc.sync.dma_start(wkv_sb[:, 1:2, :], wkvr[:, 1:2, :])
        nc.sync.dma_start(wq_sb[:, 2:4, :], wqr[:, 2:4, :])
        nc.sync.dma_start(wkv_sb[:, 2:4, :], wkvr[:, 2:4, :])
        nc.sync.dma_start(wq_sb[:, 4:8, :], wqr[:, 4:8, :])
        nc.sync.dma_start(wkv_sb[:, 4:8, :], wkvr[:, 4:8, :])
        nc.sync.dma_start(x0[:, 8:16, :], xr[:, 8:16, 0:2 * P])
        nc.sync.dma_start(cs_sb[:, 0:2, :], csr[:, 0:2, :])
        for cb in range(2, 4):
            s = slice(4 * cb, 4 * cb + 4)
            nc.sync.dma_start(wq_sb[:, s, :], wqr[:, s, :])
            nc.sync.dma_start(wkv_sb[:, s, :], wkvr[:, s, :])
        load_x(1)
        nc.sync.dma_start(cs_sb[:, 2:TOKCH, :], csr[:, 2:TOKCH, :])
        for t2 in range(2, 6):
            load_x(t2)

        ident = singles.tile([P, P], BF16)
        make_identity(nc, ident)
        for cval in (0.0, EPS, float(D) * EPS, EXPB):
            ccol = singles.tile([P, 1], F32, tag=f"c{cval}")
            nc.vector.memset(ccol, cval)
            nc.const_aps.aps[(F32, cval)] = ccol[:]

        # strict-upper triangular NEG (mask matmul stationary):
        # utri[p, m] = NEG if p < m else 0
        utri = singles.tile([P, P], BF16)
        nc.vector.memset(utri, 0.0)
        nc.gpsimd.affine_select(
            out=utri, in_=utri,
            compare_op=ALU.is_ge, fill=NEG,
            base=0, pattern=[[-1, P]], channel_multiplier=1,
        )

        qT = singles.tile([P, HG, T], BF16)      # [d, h, tok]
        kT = singles.tile([P, KVG, T], BF16)
        v_sb = singles.tile([P, TOKCH, KC], F16)  # [tok%128, chunk, vcol]
        yT = singles.tile([P, HG, T], BF16)

        # ================= phase A: QKV proj + RoPE + qk-norm =============
        if "A" in PHASES:
         with tc.tile_pool(name="pa", bufs=2, space="PSUM") as pps:
            NH = HG + KVG  # 10 rope heads
            h2 = D // 2
            nco = C // P

            def qkv_mm(ps, lhsT, co):
                st = dict(start=(co == 0), stop=(co == nco - 1))
                nc.tensor.matmul(ps[:, 0:512], lhsT, wq_sb[:, co, 0:512], **st)
                nc.tensor.matmul(ps[:, 512:1024], lhsT, wq_sb[:, co, 512:1024], **st)
                nc.tensor.matmul(ps[:, 1024:1536], lhsT, wkv_sb[:, co, :], **st)

            def postproc_a(t, ps):
                """Reads the qkv PSUM (frees its banks ASAP): V evac + RoPE."""
                # V: cast straight to resident token-major buffer (ACT)
                nc.scalar.copy(v_sb[:, t, :], ps[:, QC + KC:QC + 2 * KC])

                # RoPE (token-major, all 10 heads at once).
                # psum view: [P, NH, 2, h2] over q0..q7,k0,k1
                pv = ps[:, 0:QC + KC].rearrange("p (h a d) -> p h a d", h=NH, a=2)
                p1, p2 = pv[:, :, 0, :], pv[:, :, 1, :]
                r = spool.tile([P, NH, 2, h2], BF16, tag="r")
                r1, r2 = r[:, :, 0, :], r[:, :, 1, :]
                s2 = spool.tile([P, NH, h2], F32, tag="s2")
                s3 = spool.tile([P, NH, h2], F32, tag="s3")
                csx = cs_sb[:, t, None, 0:h2].to_broadcast([P, NH, h2])
                snx = cs_sb[:, t, None, h2:P].to_broadcast([P, NH, h2])
                # all four PSUM reads first: frees the qkv bank ~1us sooner
                nc.vector.tensor_mul(r1, p1, csx)
                nc.vector.tensor_mul(s2, p2, snx)
                nc.vector.tensor_mul(r2, p1, snx)
                nc.vector.tensor_mul(s3, p2, csx)
                nc.vector.tensor_sub(r1, r1, s2)
                nc.vector.tensor_add(r2, r2, s3)
                return (t, r)

            def postproc_b(ctx):
                """Norm + DMA-engine (XBAR) transpose; emitted a chunk late
                so the PE never waits on the DVE norm chain.  The transposes
                run entirely off the PE."""
                t, r = ctx
                # qk-norm factors (RoPE preserves per-head L2 norms, and it
                # is linear, so compute ss from r and scale r afterwards).
                rf = r.rearrange("p h a d -> p h (a d)")
                sq = qpool.tile([P, NH, D], BF16, tag="w")
                nc.vector.tensor_mul(sq, rf, rf)
                ss = spool.tile([P, NH], F32, tag="ss")
                nc.vector.tensor_reduce(ss, sq, axis=mybir.AxisListType.X,
                                        op=ALU.add)
                rt = spool.tile([P, NH], F32, tag="rt")
                # q heads: qsc/sqrt(ss/D+eps) == 1/sqrt(ss + D*eps)
                # (qsc = 1/sqrt(D) folded into the sqrt argument scale)
                nc.scalar.activation(rt[:, 0:HG], ss[:, 0:HG], AF.Sqrt,
                                     scale=1.0, bias=float(D) * EPS)
                nc.scalar.activation(rt[:, HG:NH], ss[:, HG:NH], AF.Sqrt,
                                     scale=1.0 / D, bias=EPS)
                rq = spool.tile([P, NH], F32, tag="rq")
                nc.vector.reciprocal(rq, rt)
                # deep ring: the XBAR transpose DMAs below read qk, and a
                # backed-up DMA queue must not race a later chunk's rewrite
                qk = qkpool.tile([P, NH, D], BF16, tag="qk")
                nc.vector.tensor_mul(
                    qk, rf, rq[:, :, None].to_broadcast([P, NH, D]))
                nc.sync.dma_start_transpose(
                    qT[:, :, t * P:(t + 1) * P], qk[:, 0:HG, :])
                nc.sync.dma_start_transpose(
                    kT[:, :, t * P:(t + 1) * P], qk[:, HG:NH, :])

            # chunks 0-1: co-outer so PE keeps pace with weight DMA arrival
            ps01 = [pps.tile([P, QC + 2 * KC], F32, tag="qkv", name=f"qkv{i}")
                    for i in range(2)]
            # chunk 1 trails chunk 0 by one co-block so postproc_a(0) (which
            # gates chunk 2's PSUM reuse) starts as soon as the last weight
            # block lands
            for co in range(nco):
                qkv_mm(ps01[0], xtiles[0][:, co, 0:P], co)
                if co > 0:
                    qkv_mm(ps01[1], xtiles[0][:, co - 1, P:2 * P], co - 1)
            pend_a = [postproc_a(0, ps01[0])]
            qkv_mm(ps01[1], xtiles[0][:, nco - 1, P:2 * P], nco - 1)
            pend_a.append(postproc_a(1, ps01[1]))

            for t in range(2, TOKCH):
                if t in (8, 12):
                    load_x(6 if t == 8 else 7)
                xtile = xtiles[t // 2][:, :, (t % 2) * P:(t % 2 + 1) * P]
                ps = pps.tile([P, QC + 2 * KC], F32, tag="qkv")  # 3 banks
                for co in range(nco):
                    qkv_mm(ps, xtile[:, co, :], co)
                # partB(t-1) BEFORE partA(t): partA waits mms(t) anyway, and
                # this lets partB(t-1) run on DVE during mms(t) instead of
                # queueing behind partA(t) (DVE FIFO head-of-line).  On the
                # last chunk, drain everything pending first so no partB
                # queues behind the final partA (the pool-close barrier
                # would wait for it).
                while len(pend_a) > (0 if t == TOKCH - 1 else 1):
                    postproc_b(pend_a.pop(0))
                pend_a.append(postproc_a(t, ps))
            postproc_b(pend_a.pop(0))

        a_stack.close()

        # ================= phase B+C: attention + interleaved out-proj ====
        if "B" in PHASES:
         with tc.tile_pool(name="wpp", bufs=1) as wpool, \
             tc.tile_pool(name="psc", bufs=PSC_BUFS, space="PSUM") as psc, \
             tc.tile_pool(name="psy", bufs=PSY_BUFS, space="PSUM") as psy, \
             tc.tile_pool(name="pso", bufs=PSO_BUFS, space="PSUM") as pso, \
             tc.tile_pool(name="pb", bufs=5) as ppool, \
             tc.tile_pool(name="sb", bufs=6) as bpool, \
             tc.tile_pool(name="rb", bufs=2) as rpool:
            # wproj tiles: loaded at start of B into SBUF freed by wq/wkv
            wpr = wp.rearrange("(hc p) c -> p hc c", p=P)
            wp_ts = []
            for ct in range(C // 512):
                wp_t = wpool.tile([P, HG, 512], BF16, tag=f"wpt{ct}")
                nc.sync.dma_start(wp_t, wpr[:, :, ct * 512:(ct + 1) * 512])
                wp_ts.append(wp_t)

            NT = T // 512  # 4 tq tiles

            # ---- global software pipeline ----
            # All PE-side work (scores, y-matmuls, per-head epilogues,
            # phase-C projection fillers) is emitted from one paced stream
            # so the PE never drains at head/tile boundaries.
            LAG_Y = LAG_Y_CFG   # scores between a score-mm and its y-mm
            LAG_E = LAG_E_CFG   # scores before the epilogue's DVE ops fire
            seq = 0       # scores emitted so far
            dq = []       # FIFO of (due_seq, closure)

            def pump(force=False):
                while dq and (force or dq[0][0] <= seq):
                    dq.pop(0)[1]()

            def c_ops(t, h0=0, h1=HG, obps=None, stash=False):
                """Phase-C work for tq tile t as a closure stream.

                With stash=True, partial sums (heads h0..h1) are staged to
                SBUF (obps); with obps given and stash=False, the staged
                partial is added back during evacuation.  This splits the
                final tile's projection so half of it overlaps attention.
                The last-head matmul + evacuation of each unit is deferred
                one unit so independent matmuls run ahead of late fins.
                """
                def close(unit):
                    ps_o, tc_, ct, u = unit
                    yield lambda: nc.tensor.matmul(
                        ps_o, yT[:, h1 - 1, tc_ * P:(tc_ + 1) * P],
                        wp_ts[ct][:, h1 - 1, :], start=False, stop=True)

                    def evac():
                        if stash:
                            nc.vector.tensor_copy(obps[u], ps_o)
                            return
                        ob = bpool.tile([P, 512], F32, tag="ob")
                        if obps is not None:
                            nc.vector.tensor_add(ob, obps[u], ps_o)
                        else:
                            nc.vector.tensor_copy(ob, ps_o)
                        nc.sync.dma_start(
                            out[tc_ * P:(tc_ + 1) * P,
                                ct * 512:(ct + 1) * 512], ob)
                    yield evac

                deferred = []
                for u in range(16):
                    tc_, ct = t * 4 + u // 4, u % 4
                    ps_o = pso.tile([P, 512], F32, tag="o")
                    for hc in range(h0, h1 - 1):
                        yield lambda ps_o=ps_o, tc_=tc_, ct=ct, hc=hc: \
                            nc.tensor.matmul(
                                ps_o, yT[:, hc, tc_ * P:(tc_ + 1) * P],
                                wp_ts[ct][:, hc, :],
                                start=(hc == h0), stop=False)
                    deferred.append((ps_o, tc_, ct, u))
                    if len(deferred) > 1:
                        yield from close(deferred.pop(0))
                yield from close(deferred.pop(0))

            cgens = []    # queue of [arm_seq, generator]; armed generators
                          # only — tile t's C work must not be pulled before
                          # tile t's last fin has been emitted (LAG_E scores)
            cdebt = 0.0   # fractional C-ops owed per score

            def pull_c(k):
                while k > 0 and cgens:
                    arm, gen = cgens[0]
                    if seq < arm:
                        return
                    try:
                        next(gen)()
                        k -= 1
                    except StopIteration:
                        cgens.pop(0)

            obps = [wpool.tile([P, 512], F32, tag=f"obp{u}", name=f"obp{u}")
                    for u in range(16)]
            for t in range(NT):
                nscores = HG * 4 * (t + 1)
                nops = 16 * 9 + (16 * 5 if t == NT - 1 else 0)
                cpace = CPACE_F * nops / nscores if t > 0 else 0.0
                for h in range(HG):
                    g = h // NREP
                    nch = 4 * (t + 1)
                    ps_y = psy.tile([P, 512], F32, tag="y")
                    pacc = ppool.tile([P, 512], F16, tag="pacc")

                    def ymm(c, pt, col0, ps_y=ps_y, pacc=pacc, nch=nch, g=g):
                        st = dict(start=(c == 0), stop=(c == nch - 1))
                        nc.tensor.matmul(ps_y[:, col0:512],
                                         v_sb[:, c, g * P:(g + 1) * P],
                                         pt[:, col0:512], **st)
                        if c == 0:
                            nc.vector.tensor_copy(pacc, pt)
                        else:
                            nc.vector.tensor_add(pacc[:, col0:512],
                                                 pacc[:, col0:512],
                                                 pt[:, col0:512])

                    for c in range(nch):
                        o = c * P - t * 512
                        col0 = max(o, 0)
                        ps_sc = psc.tile([P, 512], F32, tag="sc")
                        if o >= 0:
                            # causal mask: NEG upper triangle accumulated
                            # into the bank before the score matmul
                            nc.tensor.matmul(
                                ps_sc[:, col0:col0 + P], utri, ident,
                                start=True, stop=False)
                        nc.tensor.matmul(
                            ps_sc[:, col0:512], kT[:, g, c * P:(c + 1) * P],
                            qT[:, h, t * 512 + col0:(t + 1) * 512],
                            start=(o < 0), stop=True)
                        seq += 1
                        pump()  # before pt alloc: keeps ring WAR deps ordered
                        pt = ppool.tile([P, 512], F16, tag="pt")
                        nc.scalar.activation(pt[:, col0:512],
                                             ps_sc[:, col0:512],
                                             AF.Exp, bias=EXPB)
                        dq.append((seq + LAG_Y,
                                   lambda c=c, pt=pt, col0=col0, f=ymm: f(c, pt, col0)))
                        # paced phase-C filler ops
                        cdebt += cpace
                        npull = int(cdebt)
                        cdebt -= npull
                        pull_c(npull)

                    def par(pacc=pacc):
                        # softmax denominator: cross-partition sum of pacc,
                        # replicated to all partitions (GpSimd is idle)
                        rbb = rpool.tile([P, 512], F32, tag="rbb")
                        nc.gpsimd.partition_all_reduce(
                            rbb, pacc, channels=P,
                            reduce_op=bass_isa.ReduceOp.add)
                        return rbb

                    rbb_box = []
                    dq.append((seq, lambda box=rbb_box, p=par: box.append(p())))

                    def fin(ps_y=ps_y, t=t, h=h, rbb_box=rbb_box):
                        rbb = rbb_box[0]
                        nc.vector.reciprocal(rbb, rbb)
                        nc.vector.tensor_mul(
                            yT[:, h, t * 512:(t + 1) * 512], ps_y, rbb)
                    # cap: fin(i) must be emitted before head i+2 reuses the
                    # ps_y PSUM bank (psy ring is 2 deep), i.e. within the
                    # next head's nch scores
                    dq.append((seq + min(LAG_E, nch), fin))
                    if t == NT - 1 and h == 3:
                        # last tile: heads 0-3 of its projection can start
                        # once their fins land; stage partials to SBUF
                        cgens.append([seq + LAG_E + 1,
                                      c_ops(t, 0, 4, obps, stash=True)])
                # all heads of tile t done (once dq drains): queue C work,
                # armed only after this tile's last fin will have emitted
                pump()
                if t == NT - 1:
                    cgens.append([seq + LAG_E + 1, c_ops(t, 4, HG, obps)])
                else:
                    cgens.append([seq + LAG_E + 1, c_ops(t)])

            pump(force=True)
            while cgens:
                try:
                    next(cgens[0][1])()
                except StopIteration:
                    cgens.pop(0)

    nc.compile()
    return nc


_NC_CACHE = []


def _get_prog():
    if not _NC_CACHE:
        _NC_CACHE.append(_build())
    return _NC_CACHE[0]


def _make_in_maps(inputs):
    x, cos, sin = inputs["x"], inputs["cos"], inputs["sin"]
    wq, wk, wv, wproj = inputs["wq"], inputs["wk"], inputs["wv"], inputs["wproj"]
    bf = ml_dtypes.bfloat16
    cos2 = np.asarray(cos.reshape(T, D // 2), dtype=np.float32)
    sin2 = np.asarray(sin.reshape(T, D // 2), dtype=np.float32)
    cs2 = np.ascontiguousarray(np.hstack([cos2, sin2]))
    in_maps = []
    for core in range(8):
        b, g = core // 2, core % 2
        qs = slice(g * QC, (g + 1) * QC)
        ks = slice(g * KC, (g + 1) * KC)
        in_maps.append({
            "xt": np.ascontiguousarray(x[b].T).astype(bf),
            "wq": np.ascontiguousarray(wq[:, qs]).astype(bf),
            "wkv": np.ascontiguousarray(np.hstack([wk[:, ks], wv[:, ks]])).astype(bf),
            "wp": np.ascontiguousarray(wproj[qs, :]).astype(bf),
            "cs": cs2,
        })
    return in_maps


def kernel(x, cos, sin, wq, wk, wv, wproj):
    nc = _get_prog()
    in_maps = _make_in_maps(dict(x=x, cos=cos, sin=sin, wq=wq, wk=wk, wv=wv, wproj=wproj))
    res = run_bass_kernel_spmd(nc, in_maps, core_ids=list(range(8))).results
    outp = np.empty((B, T, C), np.float32)
    for b in range(B):
        outp[b] = res[2 * b]["out"] + res[2 * b + 1]["out"]
    return outp
